# revision 36
# baseline (speedup 1.0000x reference)
"""GCN (2-layer) + edge-dot decode on 8 TRN2 NeuronCores — v2.

Math (per GCN layer, dinv = rsqrt(indeg+1)):
    out[v] = dinv[v] * ( sum_{e: dst=v} hs[src_e] + hs[v] ) + b,  hs = dinv (.) (x @ W)

v2 structure (vs v1):
  * GEMM1 is REPLICATED: every core computes hs for ALL nodes from the full
    (host-prescaled dinv (.) x, bf16) input and writes a LOCAL hs table — the
    first AllGather is gone entirely.
  * All gather tables for aggregation are bf16 (256B rows): halves gather DMA
    packet time and makes the TensorE accumulation 1 cycle/row instead of 4.
  * Collectives (AllGather of g, then z) are emitted mid-loop so their
    transfers overlap the remaining aggregation blocks; decode is split into
    an AA group (both endpoints in the first table half, dep cc3a only) and
    the rest (dep cc3b), hiding cc3b behind AA decode work.
  * Per-core hs-table layout puts the core's own panel first so the SPMD
    program is uniform while hs_own/g_own rows stay resident in SBUF.

dma_gather indices are SIGNED int16: every gather uses a frame centered at
row 32768 (idx = row - 32768), and appends one all-positive pad round so a
trailing run of real negative indices is never dropped by the ucode.
"""

import sys
import numpy as np
from contextlib import ExitStack

sys.path.insert(0, "/opt/trn_rl_repo")

import concourse.bass as bass
import concourse.mybir as mybir
from concourse.bass_utils import run_bass_kernel_spmd
from concourse.tile import TileContext, add_dep_helper
from concourse.masks import make_identity
from concourse.library_config import mlp
from concourse.library_overlay import lower_extended_insts

N, E, L = 50000, 800000, 200000
IN, HID, OUT = 256, 128, 64
C = 8                      # cores
NP = 50176                 # padded node count = 392 blocks of 128
NPC = NP // C              # 6272 nodes per core
BPC = NPC // 128           # 49 blocks per core
FBASE = 32768              # gather frame base row (signed int16 centered)
PADIDX = NP - 1 - FBASE    # pad index -> row 50175 (zero pad-node row in every layout)
CH_MAX = 24                # max rounds per gather chunk (excl. appended pad round)
DEC_CHUNK = 16             # decode chunk rounds
SPLIT_O = 3200             # per-core first-half rows (25 blocks)
SPLIT_K = SPLIT_O // 128   # 25
HA = SPLIT_O
HB = NPC - SPLIT_O

BF16 = mybir.dt.bfloat16

CUSTOM_ISA_OPCODES = {"DMAGatherAnt", "DMAScatterAddAnt"}


def _fix_sync_waits(nc):
    """This container's walrus accepts at most one sync-wait per instruction
    and none on custom ISA ucode ops; hoist extras onto preceding drains."""
    f = nc.m.functions[0]
    for b in f.blocks:
        insts = b.instructions
        i = 0
        while i < len(insts):
            ins = insts[i]
            si = ins.sync_info
            nw = len(si.on_wait) if (si is not None and si.on_wait is not None) else 0
            keep = 0 if str(ins.opcode) in CUSTOM_ISA_OPCODES else 1
            if nw > keep:
                waits = list(si.on_wait)
                hoist, keepw = waits[: nw - keep], waits[nw - keep:]
                for j, w in enumerate(hoist):
                    d = mybir.InstEventSemaphore(name=f"{ins.name}-wsplit{j}")
                    d.engine = ins.engine
                    d.sync_info = mybir.SyncInfo(on_wait=[w], on_update=[])
                    insts.insert(i + j, d)
                si.on_wait = keepw
                i += len(hoist)
            i += 1


def _sortedpos(p):
    """final position -> position in the degree-sorted sequence."""
    core = p // NPC
    k = (p % NPC) // 128
    lane = p % 128
    return 128 * (8 * k + core) + lane


SPLIT_O2 = 5248            # zone B1 = blocks 25..40, B2 = blocks 41..48
HB1 = SPLIT_O2 - SPLIT_O   # 2048 rows (16 blocks)
HB2 = NPC - SPLIT_O2       # 1024 rows (8 blocks)


def _rowmap(p):
    """final position -> GLOBAL table row (tab2 / ztab), three zones:
    [cores' A (o<3200)][cores' B1 (3200..5248)][cores' B2 (5248..)]."""
    c = p // NPC
    o = p % NPC
    return np.where(o < SPLIT_O, c * SPLIT_O + o,
           np.where(o < SPLIT_O2,
                    C * SPLIT_O + c * HB1 + (o - SPLIT_O),
                    C * SPLIT_O2 + c * HB2 + (o - SPLIT_O2)))


def _wrap_idx(flat):
    """[n] int16 -> [128, n//16] wrapped in 16 partitions, replicated x8."""
    n = flat.shape[0]
    arr = np.empty((16, n // 16), dtype=np.int16)
    arr[:, :] = flat.reshape(n // 16, 16).T
    return np.tile(arr, (8, 1))


def _chunked(total, chmax):
    out = []
    r = 0
    while r < total:
        ch = min(chmax, total - r)
        out.append((r, ch))
        r += ch
    return out


def _prepare(edge_index, edge_label_index):
    src = np.asarray(edge_index[0], dtype=np.int64)
    dst = np.asarray(edge_index[1], dtype=np.int64)
    la = np.asarray(edge_label_index[0], dtype=np.int64)
    lb = np.asarray(edge_label_index[1], dtype=np.int64)

    deg = np.bincount(dst, minlength=N).astype(np.int64)

    # permutation: degree-sorted, core-striped; 176 zero pad nodes at the tail
    sorted_real = np.argsort(-deg, kind="stable")
    seq = np.full(NP, -1, dtype=np.int64)
    seq[:N] = sorted_real
    final_perm = seq[_sortedpos(np.arange(NP))]   # final position -> orig (-1 pad)
    real_mask = final_perm >= 0
    invpos = np.full(N, -1, dtype=np.int64)
    invpos[final_perm[real_mask]] = np.nonzero(real_mask)[0]
    assert final_perm[NP - 1] == -1

    # per-core hs-table layout: own panel first, LANE-MAJOR:
    # row = lane*392 + (panel*49 + k). Lane-major makes the phase-A write
    # descriptors nb*256B contiguous per partition instead of 256B.
    NBLK = NP // 128
    panel_orders = []
    rowHS = np.empty((C, NP), dtype=np.int64)
    pos = np.arange(NP)
    for c in range(C):
        porder = [c] + [d for d in range(C) if d != c]
        panel_orders.append(porder)
        panel_of = np.empty(C, dtype=np.int64)
        for j, pc in enumerate(porder):
            panel_of[pc] = j
        gblk = panel_of[pos // NPC] * BPC + (pos % NPC) // 128
        rowHS[c] = (pos % 128) * NBLK + gblk
        # row NP-1 must be a pad node in this layout:
        lastpos = np.nonzero(rowHS[c] == NP - 1)[0][0]
        assert final_perm[lastpos] == -1

    ps = invpos[src]
    pd = invpos[dst]

    # per-node in-edge ranks (dst-major)
    order = np.argsort(pd, kind="stable")
    pd_s = pd[order]
    ps_s = ps[order]
    newgrp = np.empty(E, dtype=bool)
    newgrp[0] = True
    newgrp[1:] = pd_s[1:] != pd_s[:-1]
    gidx = np.nonzero(newgrp)[0]
    rank = np.arange(E) - gidx[np.cumsum(newgrp) - 1]

    lane = pd_s % 128
    core = pd_s // NPC
    slot = (pd_s % NPC) // 128

    nblocks = NP // 128
    KB = np.zeros(nblocks, dtype=np.int64)
    np.maximum.at(KB, pd_s // 128, rank + 1)
    Khat = np.zeros(BPC, dtype=np.int64)
    for k in range(BPC):
        Khat[k] = int(KB[[c * BPC + k for c in range(C)]].max())
    off = np.zeros(BPC + 1, dtype=np.int64)
    off[1:] = np.cumsum(Khat)

    # [core, round-slot, lane] source tables for phase B (per-core rowHS) and
    # phase C (global rowmap); same rank schedule for both.
    idxB = np.full((C, off[-1], 128), PADIDX, dtype=np.int16)
    idxC = np.full((C, off[-1], 128), PADIDX, dtype=np.int16)
    gslot = off[slot] + rank
    idxC[core, gslot, lane] = (_rowmap(ps_s) - FBASE).astype(np.int16)
    for c in range(C):
        m = core == c
        idxB[c, gslot[m], lane[m]] = (rowHS[c][ps_s[m]] - FBASE).astype(np.int16)

    # chunk schedule per block (shared by B and C)
    chunks = []   # (k, r0, ch)
    for k in range(BPC):
        for (r0, ch) in _chunked(int(Khat[k]), CH_MAX):
            chunks.append((k, r0, ch))

    # degp per core [128, BPC]
    degp = np.full(NP, 1e30, dtype=np.float32)
    degp[real_mask] = (deg[final_perm[real_mask]] + 1).astype(np.float32)
    degp_core = degp.reshape(C, BPC, 128).transpose(0, 2, 1).copy()

    # ---- decode: AA group (both endpoints in first table half) then rest ----
    pa = invpos[la]
    pb = invpos[lb]
    LPC = L // C
    isAA = ((pa % NPC) < SPLIT_O) & ((pb % NPC) < SPLIT_O)
    aa_lists, rest_lists = [], []
    for c in range(C):
        s = slice(c * LPC, (c + 1) * LPC)
        aa_lists.append(np.nonzero(isAA[s])[0])
        rest_lists.append(np.nonzero(~isAA[s])[0])
    R_AA = (max(len(a) for a in aa_lists) + 127) // 128
    R_RE = (max(len(r) for r in rest_lists) + 127) // 128
    aa_chunks = _chunked(R_AA, DEC_CHUNK)
    re_chunks = _chunked(R_RE, DEC_CHUNK)

    # per-core slot map: label i (within core slice) -> flat output slot
    dec_slots = []
    for c in range(C):
        sl = np.empty(LPC, dtype=np.int64)
        sl[aa_lists[c]] = np.arange(len(aa_lists[c]))
        sl[rest_lists[c]] = R_AA * 128 + np.arange(len(rest_lists[c]))
        dec_slots.append(sl)

    padrow = np.full(128, PADIDX, dtype=np.int16)

    def dec_group_tables(c, lst, rounds, chs):
        s0 = c * LPC
        fa = np.full(rounds * 128, PADIDX, dtype=np.int16)
        fb = np.full(rounds * 128, PADIDX, dtype=np.int16)
        fa[:len(lst)] = (_rowmap(pa[s0 + lst]) - FBASE).astype(np.int16)
        fb[:len(lst)] = (_rowmap(pb[s0 + lst]) - FBASE).astype(np.int16)
        parts = []
        for (r0, ch) in chs:
            parts.append(_wrap_idx(np.concatenate([fa[r0 * 128:(r0 + ch) * 128], padrow])))
            parts.append(_wrap_idx(np.concatenate([fb[r0 * 128:(r0 + ch) * 128], padrow])))
        return parts

    idx16 = []
    for c in range(C):
        parts = []
        for (k, r0, ch) in chunks:
            flat = idxB[c, off[k] + r0: off[k] + r0 + ch].reshape(-1)
            parts.append(_wrap_idx(np.concatenate([flat, padrow])))
        for (k, r0, ch) in chunks:
            flat = idxC[c, off[k] + r0: off[k] + r0 + ch].reshape(-1)
            parts.append(_wrap_idx(np.concatenate([flat, padrow])))
        parts += dec_group_tables(c, aa_lists[c], R_AA, aa_chunks)
        parts += dec_group_tables(c, rest_lists[c], R_RE, re_chunks)
        idx16.append(np.ascontiguousarray(np.concatenate(parts, axis=1)))

    return dict(
        final_perm=final_perm, invpos=invpos, real_mask=real_mask,
        Khat=Khat, off=off, chunks=chunks,
        aa_chunks=aa_chunks, re_chunks=re_chunks, R_AA=R_AA, R_RE=R_RE,
        dec_slots=dec_slots, degp=degp, degp_core=degp_core, idx16=idx16,
        panel_orders=panel_orders,
    )


def _build(prep):
    chunks = prep["chunks"]
    aa_chunks = prep["aa_chunks"]
    re_chunks = prep["re_chunks"]
    TOTW = prep["idx16"][0].shape[1]
    ndec_cols = sum(ch for (_, ch) in aa_chunks) + sum(ch for (_, ch) in re_chunks)

    nc = bass.Bass(num_devices=C, dynamic_dma_scratch_size=32768, num_swdge_queues=4)
    f32 = mybir.dt.float32
    xT_d = nc.dram_tensor("xT", [IN, NP], BF16, kind="ExternalInput")
    W1_d = nc.dram_tensor("W1", [IN, HID], BF16, kind="ExternalInput")
    b1_d = nc.dram_tensor("b1", [1, HID], f32, kind="ExternalInput")
    W2_d = nc.dram_tensor("W2", [HID, OUT], BF16, kind="ExternalInput")
    b2_d = nc.dram_tensor("b2", [1, OUT], f32, kind="ExternalInput")
    degp_d = nc.dram_tensor("degp", [128, BPC], f32, kind="ExternalInput")
    idx_d = nc.dram_tensor("idx16", [128, TOTW], mybir.dt.int16, kind="ExternalInput")
    out_d = nc.dram_tensor("out", [128, ndec_cols], f32, kind="ExternalOutput")

    tabHS = nc.dram_tensor("tabHS", [NP, HID], BF16)
    ag2_in = nc.dram_tensor("ag2_in", [NPC, HID], BF16)
    tab2 = nc.dram_tensor("tab2", [NP, HID], BF16)
    ag3_in = nc.dram_tensor("ag3_in", [NPC, OUT], f32)
    ztab = nc.dram_tensor("ztab", [NP, OUT], f32)

    with TileContext(nc) as tc, ExitStack() as ctx:
        const = ctx.enter_context(tc.tile_pool(name="const", bufs=1))
        own = ctx.enter_context(tc.tile_pool(name="own", bufs=1))
        xp_pool = ctx.enter_context(tc.tile_pool(name="xpan", bufs=2))
        hsb = ctx.enter_context(tc.tile_pool(name="hstage", bufs=2))
        gp = ctx.enter_context(tc.tile_pool(name="gath", bufs=5))
        dgp = ctx.enter_context(tc.tile_pool(name="dgath", bufs=4))
        ppa = ctx.enter_context(tc.tile_pool(name="psA", bufs=2, space="PSUM"))
        pp = ctx.enter_context(tc.tile_pool(name="psAgg", bufs=3, space="PSUM"))
        pz = ctx.enter_context(tc.tile_pool(name="psZ", bufs=1, space="PSUM"))
        sp_ = ctx.enter_context(tc.tile_pool(name="stage", bufs=4))
        prp = ctx.enter_context(tc.tile_pool(name="prodp", bufs=1))

        ll = nc.gpsimd.load_library(mlp)

        ident = const.tile([128, 128], BF16)
        make_identity(nc, ident[:])
        identf = const.tile([128, 128], mybir.dt.float32)
        make_identity(nc, identf[:])

        idx_sb = const.tile([128, TOTW], mybir.dt.int16)
        idma = nc.sync.dma_start(out=idx_sb[:], in_=idx_d[:, :])
        add_dep_helper(idma.ins, ll.ins, reason="idx after lib load")

        kvals = sorted({(ch + 1) * 128 for (_, _, ch) in chunks}
                       | {(ch + 1) * 128 for (_, ch) in aa_chunks}
                       | {(ch + 1) * 128 for (_, ch) in re_chunks})
        kreg = {}
        for v in kvals:
            r = ctx.enter_context(nc.gpsimd.register(f"nidx{v}"))
            nc.gpsimd.reg_mov(r, v)
            kreg[v] = r

        W1_sb = []
        for i in range(2):
            w1t = const.tile([128, HID], BF16, tag=f"w1_{i}", name=f"w1_{i}")
            nc.sync.dma_start(out=w1t[:], in_=W1_d[i * 128:(i + 1) * 128, :])
            W1_sb.append(w1t)
        W2_sb = const.tile([128, OUT], BF16)
        nc.sync.dma_start(out=W2_sb[:], in_=W2_d[:, :])

        ones_row = const.tile([1, 128], f32)
        nc.vector.memset(ones_row[:], 1.0)
        b1_row = const.tile([1, HID], f32)
        nc.sync.dma_start(out=b1_row[:], in_=b1_d[:, :])
        b2_row = const.tile([1, OUT], f32)
        nc.sync.dma_start(out=b2_row[:], in_=b2_d[:, :])
        bias1 = const.tile([128, HID], f32)
        bps = pz.tile([128, HID], f32, tag="qt")
        nc.tensor.matmul(out=bps[:], lhsT=ones_row[:], rhs=b1_row[:], start=True, stop=True)
        nc.scalar.activation(out=bias1[:], in_=bps[:], func=mybir.ActivationFunctionType.Copy)
        bias2 = const.tile([128, OUT], f32)
        bps2 = pz.tile([128, OUT], f32, tag="qt")
        nc.tensor.matmul(out=bps2[:], lhsT=ones_row[:], rhs=b2_row[:], start=True, stop=True)
        nc.scalar.activation(out=bias2[:], in_=bps2[:], func=mybir.ActivationFunctionType.Copy)

        degp_sb = const.tile([128, BPC], f32)
        nc.sync.dma_start(out=degp_sb[:], in_=degp_d[:, :])
        rec = const.tile([128, BPC], f32)
        nc.vector.reciprocal(out=rec[:], in_=degp_sb[:])
        dinv = const.tile([128, BPC], f32)
        nc.scalar.activation(out=dinv[:], in_=rec[:], func=mybir.ActivationFunctionType.Sqrt)

        hs_own = own.tile([128, NPC], BF16)
        g_own = own.tile([128, NPC], BF16)
        z_own = own.tile([128, NPC // 2], f32)   # 49 blocks x 64 cols

        # chunk -> idx column offsets (B group, C group, decode groups)
        co = 0
        blk_chunksB = [[] for _ in range(BPC)]
        for (k, r0, ch) in chunks:
            blk_chunksB[k].append((co, ch))
            co += (ch + 1) * 8
        blk_chunksC = [[] for _ in range(BPC)]
        for (k, r0, ch) in chunks:
            blk_chunksC[k].append((co, ch))
            co += (ch + 1) * 8
        aa_coffs = []
        for (r0, ch) in aa_chunks:
            aa_coffs.append(co)          # a-cols, then b-cols adjacent
            co += 2 * (ch + 1) * 8
        re_coffs = []
        for (r0, ch) in re_chunks:
            re_coffs.append(co)
            co += 2 * (ch + 1) * 8
        assert co == TOTW

        # ---------------- Phase A: replicated GEMM1 -> local hs table ----------------
        wrA = []
        with nc.named_scope("gemm1"):
            for j in range(C):                      # panel j: tabHS rows [j*NPC, (j+1)*NPC)
                for (o0, nb) in ((0, SPLIT_K), (SPLIT_O, BPC - SPLIT_K)):
                    x0 = xp_pool.tile([128, SPLIT_O], BF16, tag="x0", name="x0")
                    nc.sync.dma_start(out=x0[:, :nb * 128],
                                      in_=xT_d[0:128, j * NPC + o0: j * NPC + o0 + nb * 128])
                    x1 = xp_pool.tile([128, SPLIT_O], BF16, tag="x1", name="x1")
                    nc.sync.dma_start(out=x1[:, :nb * 128],
                                      in_=xT_d[128:256, j * NPC + o0: j * NPC + o0 + nb * 128])
                    if j == 0:
                        hpan = None
                    else:
                        hpan = hsb.tile([128, SPLIT_O], BF16, tag="hst", name="hst")
                    for q0 in range(0, nb, 4):
                        G = min(4, nb - q0)
                        ps = ppa.tile([128, 512], f32, tag="a", name="psa")
                        for g in range(G):
                            cs = slice((q0 + g) * 128, (q0 + g + 1) * 128)
                            nc.tensor.matmul(out=ps[:, g * 128:(g + 1) * 128],
                                             lhsT=x0[:, cs], rhs=W1_sb[0][:],
                                             start=True, stop=False)
                            nc.tensor.matmul(out=ps[:, g * 128:(g + 1) * 128],
                                             lhsT=x1[:, cs], rhs=W1_sb[1][:],
                                             start=False, stop=True)
                        if j == 0:
                            dst = hs_own[:, o0 + q0 * 128: o0 + (q0 + G) * 128]
                        else:
                            dst = hpan[:, q0 * 128:(q0 + G) * 128]
                        if (q0 // 4) % 2 == 0:
                            nc.scalar.activation(out=dst, in_=ps[:, :G * 128],
                                                 func=mybir.ActivationFunctionType.Copy)
                        else:
                            nc.vector.tensor_copy(out=dst, in_=ps[:, :G * 128])
                    # lane-major write: rows lane*NBLK + [b0, b0+nb)
                    b0 = j * BPC + o0 // 128
                    srct = hs_own[:, o0:o0 + nb * 128] if j == 0 else hpan[:, :nb * 128]
                    wrA.append(nc.sync.dma_start(
                        out=tabHS.rearrange("(l b) f -> l b f", b=NP // 128)[
                            :, b0:b0 + nb, :],
                        in_=srct.rearrange("p (b f) -> p b f", f=HID)))

        qctr = [0]

        def aggregate(k, tab, blk_chunks, own_tile, dep_ins):
            ps = pp.tile([128, HID], f32, tag="main", name="psagg")
            first = True
            for ci, (coff, ch) in enumerate(blk_chunks[k]):
                gt = gp.tile([128, CH_MAX + 1, HID], BF16, tag="gt", name="gt")
                qctr[0] = (qctr[0] + 1) % 4
                gi = nc.gpsimd.dma_gather(
                    gt[:, :ch + 1, :], tab[FBASE:, :],
                    idx_sb[:, coff:coff + (ch + 1) * 8],
                    (ch + 1) * 128, kreg[(ch + 1) * 128], HID, single_packet=False,
                    queue_num=qctr[0])
                for d in dep_ins:
                    add_dep_helper(gi.ins, d.ins, reason="gather after table ready")
                for r in range(ch):
                    nc.tensor.matmul(out=ps[:], lhsT=ident[:], rhs=gt[:, r, :],
                                     start=first, stop=False)
                    first = False
            nc.tensor.matmul(out=ps[:], lhsT=ident[:],
                             rhs=own_tile[:, k * 128:(k + 1) * 128],
                             start=first, stop=True)
            return ps

        # ---------------- Phase B: layer-1 aggregation (local) -> g ----------------
        wr2 = []
        ccs2 = []
        with nc.named_scope("agg1"):
            for k in range(BPC):
                dep = wrA if k == 0 else []
                ps = aggregate(k, tabHS, blk_chunksB, hs_own, dep)
                t1 = sp_.tile([128, HID], f32, tag="t1", name="t1")
                nc.scalar.activation(out=t1[:], in_=ps[:],
                                     func=mybir.ActivationFunctionType.Copy,
                                     scale=dinv[:, k:k + 1])
                t2 = sp_.tile([128, HID], f32, tag="t2", name="t2")
                nc.vector.tensor_add(out=t2[:], in0=t1[:], in1=bias1[:])
                nc.scalar.activation(out=g_own[:, k * 128:(k + 1) * 128], in_=t2[:],
                                     func=mybir.ActivationFunctionType.Relu,
                                     scale=dinv[:, k:k + 1])
                if k == SPLIT_K - 1:
                    w2a = nc.sync.dma_start(
                        out=ag2_in[0:HA, :].rearrange("(b l) f -> l b f", l=128),
                        in_=g_own[:, :HA].rearrange("p (b f) -> p b f", f=HID))
                    with nc.named_scope("ag2a"):
                        cc = nc.gpsimd.collective_compute(
                            "AllGather", mybir.AluOpType.bypass,
                            replica_groups=[list(range(C))],
                            ins=[ag2_in[0:HA, :].opt()], outs=[tab2[0:C * HA, :].opt()])
                        add_dep_helper(cc.ins, w2a.ins, reason="ag2a")
                        ccs2.append(cc)
            w2b = nc.sync.dma_start(
                out=ag2_in[HA:NPC, :].rearrange("(b l) f -> l b f", l=128),
                in_=g_own[:, HA:NPC].rearrange("p (b f) -> p b f", f=HID))
            with nc.named_scope("ag2b"):
                cc = nc.gpsimd.collective_compute(
                    "AllGather", mybir.AluOpType.bypass,
                    replica_groups=[list(range(C))],
                    ins=[ag2_in[HA:SPLIT_O2, :].opt()],
                    outs=[tab2[C * HA:C * SPLIT_O2, :].opt()])
                add_dep_helper(cc.ins, w2b.ins, reason="ag2b1")
                ccs2.append(cc)
                cc = nc.gpsimd.collective_compute(
                    "AllGather", mybir.AluOpType.bypass,
                    replica_groups=[list(range(C))],
                    ins=[ag2_in[SPLIT_O2:NPC, :].opt()],
                    outs=[tab2[C * SPLIT_O2:NP, :].opt()])
                add_dep_helper(cc.ins, w2b.ins, reason="ag2b2")
                ccs2.append(cc)

        # ---------------- Phase C: layer-2 aggregation + GEMM2 -> z ----------------
        wr3 = []
        ccs3 = []
        with nc.named_scope("agg2"):
            for k in range(BPC):
                dep = ccs2 if k == 0 else []
                ps = aggregate(k, tab2, blk_chunksC, g_own, dep)
                q = sp_.tile([128, HID], f32, tag="q", name="q")
                nc.scalar.activation(out=q[:], in_=ps[:],
                                     func=mybir.ActivationFunctionType.Copy,
                                     scale=dinv[:, k:k + 1])
                qt_ps = pz.tile([128, HID], f32, tag="qt", name="qtps")
                nc.tensor.transpose(out=qt_ps[:], in_=q[:], identity=identf[:])
                qt = sp_.tile([128, HID], BF16, tag="qt_sb", name="qtsb")
                nc.vector.tensor_copy(out=qt[:], in_=qt_ps[:])
                zps = pz.tile([128, OUT], f32, tag="z", name="zps")
                nc.tensor.matmul(out=zps[:], lhsT=qt[:], rhs=W2_sb[:], start=True, stop=True)
                nc.vector.tensor_add(out=z_own[:, k * OUT:(k + 1) * OUT],
                                     in0=zps[:], in1=bias2[:])
                if k == SPLIT_K - 1:
                    w3a = nc.sync.dma_start(
                        out=ag3_in[0:HA, :].rearrange("(b l) f -> l b f", l=128),
                        in_=z_own[:, :SPLIT_K * OUT].rearrange("p (b f) -> p b f", f=OUT))
                    with nc.named_scope("ag3a"):
                        cc = nc.gpsimd.collective_compute(
                            "AllGather", mybir.AluOpType.bypass,
                            replica_groups=[list(range(C))],
                            ins=[ag3_in[0:HA, :].opt()], outs=[ztab[0:C * HA, :].opt()])
                        add_dep_helper(cc.ins, w3a.ins, reason="ag3a")
                        ccs3.append(cc)
                if k == 40:
                    w3b1 = nc.sync.dma_start(
                        out=ag3_in[HA:SPLIT_O2, :].rearrange("(b l) f -> l b f", l=128),
                        in_=z_own[:, SPLIT_K * OUT:41 * OUT].rearrange(
                            "p (b f) -> p b f", f=OUT))
                    with nc.named_scope("ag3b1"):
                        cc = nc.gpsimd.collective_compute(
                            "AllGather", mybir.AluOpType.bypass,
                            replica_groups=[list(range(C))],
                            ins=[ag3_in[HA:SPLIT_O2, :].opt()],
                            outs=[ztab[C * HA:C * SPLIT_O2, :].opt()])
                        add_dep_helper(cc.ins, w3b1.ins, reason="ag3b1")
                        ccs3.append(cc)
            w3b2 = nc.sync.dma_start(
                out=ag3_in[SPLIT_O2:NPC, :].rearrange("(b l) f -> l b f", l=128),
                in_=z_own[:, 41 * OUT:].rearrange("p (b f) -> p b f", f=OUT))
            with nc.named_scope("ag3b"):
                cc = nc.gpsimd.collective_compute(
                    "AllGather", mybir.AluOpType.bypass,
                    replica_groups=[list(range(C))],
                    ins=[ag3_in[SPLIT_O2:NPC, :].opt()],
                    outs=[ztab[C * SPLIT_O2:NP, :].opt()])
                add_dep_helper(cc.ins, w3b2.ins, reason="ag3b2")
                ccs3.append(cc)

        # ---------------- Phase D: decode (AA after ag3a; rest after ag3b) ----------------
        with nc.named_scope("decode"):
            out_sb = own.tile([128, ndec_cols], f32)
            col = 0
            for grp_chunks, grp_coffs, grp_dep in (
                    (aa_chunks, aa_coffs, [ccs3[0]]),
                    (re_chunks, re_coffs, ccs3)):
                for i, (r0, ch) in enumerate(grp_chunks):
                    coff = grp_coffs[i]
                    qa = (2 * i) % 4
                    qb = (2 * i + 1) % 4
                    za = dgp.tile([128, DEC_CHUNK + 1, OUT], f32, tag="za", name="za")
                    ga = nc.gpsimd.dma_gather(
                        za[:, :ch + 1, :], ztab[FBASE:, :],
                        idx_sb[:, coff:coff + (ch + 1) * 8],
                        (ch + 1) * 128, kreg[(ch + 1) * 128], OUT, single_packet=False,
                        queue_num=qa)
                    zb = dgp.tile([128, DEC_CHUNK + 1, OUT], f32, tag="zb", name="zb")
                    gb = nc.gpsimd.dma_gather(
                        zb[:, :ch + 1, :], ztab[FBASE:, :],
                        idx_sb[:, coff + (ch + 1) * 8:coff + 2 * (ch + 1) * 8],
                        (ch + 1) * 128, kreg[(ch + 1) * 128], OUT, single_packet=False,
                        queue_num=qb)
                    for cc in grp_dep:
                        add_dep_helper(ga.ins, cc.ins, reason="decode after AG3")
                        add_dep_helper(gb.ins, cc.ins, reason="decode after AG3")
                    prod = prp.tile([128, ch * OUT], f32, tag="prod", name="prod")
                    nc.vector.tensor_mul(out=prod[:].rearrange("p (c o) -> p c o", o=OUT),
                                         in0=za[:, :ch, :], in1=zb[:, :ch, :])
                    nc.vector.reduce_sum(out=out_sb[:, col:col + ch],
                                         in_=prod[:].rearrange("p (c o) -> p c o", o=OUT),
                                         axis=mybir.AxisListType.X)
                    col += ch
            nc.sync.dma_start(out=out_d[:, :], in_=out_sb[:])

    lower_extended_insts(nc)
    _fix_sync_waits(nc)
    return nc


def kernel(x, W1, b1, W2, b2, edge_index, edge_label_index):
    x = np.asarray(x, dtype=np.float32)
    W1 = np.asarray(W1, dtype=np.float32)
    b1 = np.asarray(b1, dtype=np.float32)
    W2 = np.asarray(W2, dtype=np.float32)
    b2 = np.asarray(b2, dtype=np.float32)
    prep = _prepare(np.asarray(edge_index), np.asarray(edge_label_index))
    nc = _build(prep)

    # host-prescaled x: xp[pos] = dinv[pos] * x[perm[pos]]  (pads stay zero)
    xp = np.zeros((NP, IN), dtype=np.float32)
    rm = prep["real_mask"]
    xp[rm] = x[prep["final_perm"][rm]]
    dinv_full = 1.0 / np.sqrt(prep["degp"])
    xp *= dinv_full[:, None]

    import ml_dtypes

    def to_bf16(a):
        return np.asarray(a, dtype=np.float32).astype(ml_dtypes.bfloat16)

    in_maps = []
    for c in range(C):
        porder = prep["panel_orders"][c]
        xc = np.concatenate([xp[pc * NPC:(pc + 1) * NPC] for pc in porder], axis=0)
        in_maps.append({
            "xT": to_bf16(np.ascontiguousarray(xc.T)),
            "W1": to_bf16(W1), "b1": b1.reshape(1, HID),
            "W2": to_bf16(W2), "b2": b2.reshape(1, OUT),
            "degp": prep["degp_core"][c],
            "idx16": prep["idx16"][c],
        })
    res = run_bass_kernel_spmd(nc, in_maps, core_ids=list(range(C)))

    LPC = L // C
    out = np.empty(L, dtype=np.float32)
    for c in range(C):
        o = res.results[c]["out"]          # [128, ncols]; slot s at (s%128, s//128)
        sl = prep["dec_slots"][c]
        out[c * LPC:(c + 1) * LPC] = o[sl % 128, sl // 128]
    return out


# revision 37
# speedup vs baseline: 1.0243x; 1.0243x over previous
"""GCN (2-layer) + edge-dot decode on 8 TRN2 NeuronCores — v2.

Math (per GCN layer, dinv = rsqrt(indeg+1)):
    out[v] = dinv[v] * ( sum_{e: dst=v} hs[src_e] + hs[v] ) + b,  hs = dinv (.) (x @ W)

v2 structure (vs v1):
  * GEMM1 is REPLICATED: every core computes hs for ALL nodes from the full
    (host-prescaled dinv (.) x, bf16) input and writes a LOCAL hs table — the
    first AllGather is gone entirely.
  * All gather tables for aggregation are bf16 (256B rows): halves gather DMA
    packet time and makes the TensorE accumulation 1 cycle/row instead of 4.
  * Collectives (AllGather of g, then z) are emitted mid-loop so their
    transfers overlap the remaining aggregation blocks; decode is split into
    an AA group (both endpoints in the first table half, dep cc3a only) and
    the rest (dep cc3b), hiding cc3b behind AA decode work.
  * Per-core hs-table layout puts the core's own panel first so the SPMD
    program is uniform while hs_own/g_own rows stay resident in SBUF.

dma_gather indices are SIGNED int16: every gather uses a frame centered at
row 32768 (idx = row - 32768), and appends one all-positive pad round so a
trailing run of real negative indices is never dropped by the ucode.
"""

import sys
import numpy as np
from contextlib import ExitStack

sys.path.insert(0, "/opt/trn_rl_repo")

import concourse.bass as bass
import concourse.mybir as mybir
from concourse.bass_utils import run_bass_kernel_spmd
from concourse.tile import TileContext, add_dep_helper
from concourse.masks import make_identity
from concourse.library_config import mlp
from concourse.library_overlay import lower_extended_insts

N, E, L = 50000, 800000, 200000
IN, HID, OUT = 256, 128, 64
C = 8                      # cores
NP = 50176                 # padded node count = 392 blocks of 128
NPC = NP // C              # 6272 nodes per core
BPC = NPC // 128           # 49 blocks per core
FBASE = 32768              # gather frame base row (signed int16 centered)
PADIDX = NP - 1 - FBASE    # pad index -> row 50175 (zero pad-node row in every layout)
CH_MAX = 24                # max rounds per gather chunk (excl. appended pad round)
DEC_CHUNK = 16             # decode chunk rounds
SPLIT_O = 3200             # per-core first-half rows (25 blocks)
SPLIT_K = SPLIT_O // 128   # 25
HA = SPLIT_O
HB = NPC - SPLIT_O

BF16 = mybir.dt.bfloat16

CUSTOM_ISA_OPCODES = {"DMAGatherAnt", "DMAScatterAddAnt"}


def _fix_sync_waits(nc):
    """This container's walrus accepts at most one sync-wait per instruction
    and none on custom ISA ucode ops; hoist extras onto preceding drains."""
    f = nc.m.functions[0]
    for b in f.blocks:
        insts = b.instructions
        i = 0
        while i < len(insts):
            ins = insts[i]
            si = ins.sync_info
            nw = len(si.on_wait) if (si is not None and si.on_wait is not None) else 0
            keep = 0 if str(ins.opcode) in CUSTOM_ISA_OPCODES else 1
            if nw > keep:
                waits = list(si.on_wait)
                hoist, keepw = waits[: nw - keep], waits[nw - keep:]
                for j, w in enumerate(hoist):
                    d = mybir.InstEventSemaphore(name=f"{ins.name}-wsplit{j}")
                    d.engine = ins.engine
                    d.sync_info = mybir.SyncInfo(on_wait=[w], on_update=[])
                    insts.insert(i + j, d)
                si.on_wait = keepw
                i += len(hoist)
            i += 1


def _sortedpos(p):
    """final position -> position in the degree-sorted sequence."""
    core = p // NPC
    k = (p % NPC) // 128
    lane = p % 128
    return 128 * (8 * k + core) + lane


SPLIT_O2 = 5248            # zone B1 = blocks 25..40, B2 = blocks 41..48
HB1 = SPLIT_O2 - SPLIT_O   # 2048 rows (16 blocks)
HB2 = NPC - SPLIT_O2       # 1024 rows (8 blocks)


def _rowmap(p):
    """final position -> GLOBAL table row (tab2 / ztab), three zones:
    [cores' A (o<3200)][cores' B1 (3200..5248)][cores' B2 (5248..)]."""
    c = p // NPC
    o = p % NPC
    return np.where(o < SPLIT_O, c * SPLIT_O + o,
           np.where(o < SPLIT_O2,
                    C * SPLIT_O + c * HB1 + (o - SPLIT_O),
                    C * SPLIT_O2 + c * HB2 + (o - SPLIT_O2)))


def _wrap_idx(flat):
    """[n] int16 -> [128, n//16] wrapped in 16 partitions, replicated x8."""
    n = flat.shape[0]
    arr = np.empty((16, n // 16), dtype=np.int16)
    arr[:, :] = flat.reshape(n // 16, 16).T
    return np.tile(arr, (8, 1))


def _chunked(total, chmax):
    out = []
    r = 0
    while r < total:
        ch = min(chmax, total - r)
        out.append((r, ch))
        r += ch
    return out


def _prepare(edge_index, edge_label_index):
    src = np.asarray(edge_index[0], dtype=np.int64)
    dst = np.asarray(edge_index[1], dtype=np.int64)
    la = np.asarray(edge_label_index[0], dtype=np.int64)
    lb = np.asarray(edge_label_index[1], dtype=np.int64)

    deg = np.bincount(dst, minlength=N).astype(np.int64)

    # permutation: degree-sorted, core-striped; 176 zero pad nodes at the tail
    sorted_real = np.argsort(-deg, kind="stable")
    seq = np.full(NP, -1, dtype=np.int64)
    seq[:N] = sorted_real
    final_perm = seq[_sortedpos(np.arange(NP))]   # final position -> orig (-1 pad)
    real_mask = final_perm >= 0
    invpos = np.full(N, -1, dtype=np.int64)
    invpos[final_perm[real_mask]] = np.nonzero(real_mask)[0]
    assert final_perm[NP - 1] == -1

    # per-core hs-table layout: own panel first, LANE-MAJOR:
    # row = lane*392 + (panel*49 + k). Lane-major makes the phase-A write
    # descriptors nb*256B contiguous per partition instead of 256B.
    NBLK = NP // 128
    panel_orders = []
    rowHS = np.empty((C, NP), dtype=np.int64)
    pos = np.arange(NP)
    for c in range(C):
        porder = [c] + [d for d in range(C) if d != c]
        panel_orders.append(porder)
        panel_of = np.empty(C, dtype=np.int64)
        for j, pc in enumerate(porder):
            panel_of[pc] = j
        gblk = panel_of[pos // NPC] * BPC + (pos % NPC) // 128
        rowHS[c] = (pos % 128) * NBLK + gblk
        # row NP-1 must be a pad node in this layout:
        lastpos = np.nonzero(rowHS[c] == NP - 1)[0][0]
        assert final_perm[lastpos] == -1

    ps = invpos[src]
    pd = invpos[dst]

    # per-node in-edge ranks (dst-major)
    order = np.argsort(pd, kind="stable")
    pd_s = pd[order]
    ps_s = ps[order]
    newgrp = np.empty(E, dtype=bool)
    newgrp[0] = True
    newgrp[1:] = pd_s[1:] != pd_s[:-1]
    gidx = np.nonzero(newgrp)[0]
    rank = np.arange(E) - gidx[np.cumsum(newgrp) - 1]

    lane = pd_s % 128
    core = pd_s // NPC
    slot = (pd_s % NPC) // 128

    nblocks = NP // 128
    KB = np.zeros(nblocks, dtype=np.int64)
    np.maximum.at(KB, pd_s // 128, rank + 1)
    Khat = np.zeros(BPC, dtype=np.int64)
    for k in range(BPC):
        Khat[k] = int(KB[[c * BPC + k for c in range(C)]].max())
    off = np.zeros(BPC + 1, dtype=np.int64)
    off[1:] = np.cumsum(Khat)

    # [core, round-slot, lane] source tables for phase B (per-core rowHS) and
    # phase C (global rowmap); same rank schedule for both.
    idxB = np.full((C, off[-1], 128), PADIDX, dtype=np.int16)
    idxC = np.full((C, off[-1], 128), PADIDX, dtype=np.int16)
    gslot = off[slot] + rank
    idxC[core, gslot, lane] = (_rowmap(ps_s) - FBASE).astype(np.int16)
    for c in range(C):
        m = core == c
        idxB[c, gslot[m], lane[m]] = (rowHS[c][ps_s[m]] - FBASE).astype(np.int16)

    # chunk schedule per block (shared by B and C)
    chunks = []   # (k, r0, ch)
    for k in range(BPC):
        for (r0, ch) in _chunked(int(Khat[k]), CH_MAX):
            chunks.append((k, r0, ch))

    # degp per core [128, BPC]
    degp = np.full(NP, 1e30, dtype=np.float32)
    degp[real_mask] = (deg[final_perm[real_mask]] + 1).astype(np.float32)
    degp_core = degp.reshape(C, BPC, 128).transpose(0, 2, 1).copy()

    # ---- decode: AA group (both endpoints in first table half) then rest ----
    pa = invpos[la]
    pb = invpos[lb]
    LPC = L // C
    isAA = ((pa % NPC) < SPLIT_O) & ((pb % NPC) < SPLIT_O)
    aa_lists, rest_lists = [], []
    for c in range(C):
        s = slice(c * LPC, (c + 1) * LPC)
        aa_lists.append(np.nonzero(isAA[s])[0])
        rest_lists.append(np.nonzero(~isAA[s])[0])
    R_AA = (max(len(a) for a in aa_lists) + 127) // 128
    R_RE = (max(len(r) for r in rest_lists) + 127) // 128
    aa_chunks = _chunked(R_AA, DEC_CHUNK)
    re_chunks = _chunked(R_RE, DEC_CHUNK)

    # per-core slot map: label i (within core slice) -> flat output slot
    dec_slots = []
    for c in range(C):
        sl = np.empty(LPC, dtype=np.int64)
        sl[aa_lists[c]] = np.arange(len(aa_lists[c]))
        sl[rest_lists[c]] = R_AA * 128 + np.arange(len(rest_lists[c]))
        dec_slots.append(sl)

    padrow = np.full(128, PADIDX, dtype=np.int16)

    def dec_group_tables(c, lst, rounds, chs):
        s0 = c * LPC
        fa = np.full(rounds * 128, PADIDX, dtype=np.int16)
        fb = np.full(rounds * 128, PADIDX, dtype=np.int16)
        fa[:len(lst)] = (_rowmap(pa[s0 + lst]) - FBASE).astype(np.int16)
        fb[:len(lst)] = (_rowmap(pb[s0 + lst]) - FBASE).astype(np.int16)
        parts = []
        for (r0, ch) in chs:
            parts.append(_wrap_idx(np.concatenate([fa[r0 * 128:(r0 + ch) * 128], padrow])))
            parts.append(_wrap_idx(np.concatenate([fb[r0 * 128:(r0 + ch) * 128], padrow])))
        return parts

    idx16 = []
    for c in range(C):
        parts = []
        for (k, r0, ch) in chunks:
            flat = idxB[c, off[k] + r0: off[k] + r0 + ch].reshape(-1)
            parts.append(_wrap_idx(np.concatenate([flat, padrow])))
        for (k, r0, ch) in chunks:
            flat = idxC[c, off[k] + r0: off[k] + r0 + ch].reshape(-1)
            parts.append(_wrap_idx(np.concatenate([flat, padrow])))
        parts += dec_group_tables(c, aa_lists[c], R_AA, aa_chunks)
        parts += dec_group_tables(c, rest_lists[c], R_RE, re_chunks)
        idx16.append(np.ascontiguousarray(np.concatenate(parts, axis=1)))

    return dict(
        final_perm=final_perm, invpos=invpos, real_mask=real_mask,
        Khat=Khat, off=off, chunks=chunks,
        aa_chunks=aa_chunks, re_chunks=re_chunks, R_AA=R_AA, R_RE=R_RE,
        dec_slots=dec_slots, degp=degp, degp_core=degp_core, idx16=idx16,
        panel_orders=panel_orders,
    )


def _build(prep):
    chunks = prep["chunks"]
    aa_chunks = prep["aa_chunks"]
    re_chunks = prep["re_chunks"]
    TOTW = prep["idx16"][0].shape[1]
    ndec_cols = sum(ch for (_, ch) in aa_chunks) + sum(ch for (_, ch) in re_chunks)

    nc = bass.Bass(num_devices=C, dynamic_dma_scratch_size=32768, num_swdge_queues=4)
    f32 = mybir.dt.float32
    xT_d = nc.dram_tensor("xT", [IN, NP], BF16, kind="ExternalInput")
    W1_d = nc.dram_tensor("W1", [IN, HID], BF16, kind="ExternalInput")
    b1_d = nc.dram_tensor("b1", [1, HID], f32, kind="ExternalInput")
    W2_d = nc.dram_tensor("W2", [HID, OUT], BF16, kind="ExternalInput")
    b2_d = nc.dram_tensor("b2", [1, OUT], f32, kind="ExternalInput")
    degp_d = nc.dram_tensor("degp", [128, BPC], f32, kind="ExternalInput")
    idx_d = nc.dram_tensor("idx16", [128, TOTW], mybir.dt.int16, kind="ExternalInput")
    out_d = nc.dram_tensor("out", [128, ndec_cols], f32, kind="ExternalOutput")

    tabHS = nc.dram_tensor("tabHS", [NP, HID], BF16)
    ag2_in = nc.dram_tensor("ag2_in", [NPC, HID], BF16)
    tab2 = nc.dram_tensor("tab2", [NP, HID], BF16)
    ag3_in = nc.dram_tensor("ag3_in", [NPC, OUT], f32)
    ztab = nc.dram_tensor("ztab", [NP, OUT], f32)

    with TileContext(nc) as tc, ExitStack() as ctx:
        const = ctx.enter_context(tc.tile_pool(name="const", bufs=1))
        own = ctx.enter_context(tc.tile_pool(name="own", bufs=1))
        xp_pool = ctx.enter_context(tc.tile_pool(name="xpan", bufs=2))
        hsb = ctx.enter_context(tc.tile_pool(name="hstage", bufs=2))
        gp = ctx.enter_context(tc.tile_pool(name="gath", bufs=5))
        dgp = ctx.enter_context(tc.tile_pool(name="dgath", bufs=3))
        ppa = ctx.enter_context(tc.tile_pool(name="psA", bufs=2, space="PSUM"))
        pp = ctx.enter_context(tc.tile_pool(name="psAgg", bufs=3, space="PSUM"))
        pz = ctx.enter_context(tc.tile_pool(name="psZ", bufs=1, space="PSUM"))
        sp_ = ctx.enter_context(tc.tile_pool(name="stage", bufs=4))
        prp = ctx.enter_context(tc.tile_pool(name="prodp", bufs=2))

        ll = nc.gpsimd.load_library(mlp)

        ident = const.tile([128, 128], BF16)
        make_identity(nc, ident[:])
        identf = const.tile([128, 128], mybir.dt.float32)
        make_identity(nc, identf[:])

        idx_sb = const.tile([128, TOTW], mybir.dt.int16)
        idma = nc.sync.dma_start(out=idx_sb[:], in_=idx_d[:, :])
        add_dep_helper(idma.ins, ll.ins, reason="idx after lib load")

        kvals = sorted({(ch + 1) * 128 for (_, _, ch) in chunks}
                       | {(ch + 1) * 128 for (_, ch) in aa_chunks}
                       | {(ch + 1) * 128 for (_, ch) in re_chunks})
        kreg = {}
        for v in kvals:
            r = ctx.enter_context(nc.gpsimd.register(f"nidx{v}"))
            nc.gpsimd.reg_mov(r, v)
            kreg[v] = r

        W1_sb = []
        for i in range(2):
            w1t = const.tile([128, HID], BF16, tag=f"w1_{i}", name=f"w1_{i}")
            nc.sync.dma_start(out=w1t[:], in_=W1_d[i * 128:(i + 1) * 128, :])
            W1_sb.append(w1t)
        W2_sb = const.tile([128, OUT], BF16)
        nc.sync.dma_start(out=W2_sb[:], in_=W2_d[:, :])

        ones_row = const.tile([1, 128], f32)
        nc.vector.memset(ones_row[:], 1.0)
        b1_row = const.tile([1, HID], f32)
        nc.sync.dma_start(out=b1_row[:], in_=b1_d[:, :])
        b2_row = const.tile([1, OUT], f32)
        nc.sync.dma_start(out=b2_row[:], in_=b2_d[:, :])
        bias1 = const.tile([128, HID], f32)
        bps = pz.tile([128, HID], f32, tag="qt")
        nc.tensor.matmul(out=bps[:], lhsT=ones_row[:], rhs=b1_row[:], start=True, stop=True)
        nc.scalar.activation(out=bias1[:], in_=bps[:], func=mybir.ActivationFunctionType.Copy)
        bias2 = const.tile([128, OUT], f32)
        bps2 = pz.tile([128, OUT], f32, tag="qt")
        nc.tensor.matmul(out=bps2[:], lhsT=ones_row[:], rhs=b2_row[:], start=True, stop=True)
        nc.scalar.activation(out=bias2[:], in_=bps2[:], func=mybir.ActivationFunctionType.Copy)

        degp_sb = const.tile([128, BPC], f32)
        nc.sync.dma_start(out=degp_sb[:], in_=degp_d[:, :])
        rec = const.tile([128, BPC], f32)
        nc.vector.reciprocal(out=rec[:], in_=degp_sb[:])
        dinv = const.tile([128, BPC], f32)
        nc.scalar.activation(out=dinv[:], in_=rec[:], func=mybir.ActivationFunctionType.Sqrt)

        hs_own = own.tile([128, NPC], BF16)
        g_own = own.tile([128, NPC], BF16)
        z_own = own.tile([128, NPC // 2], f32)   # 49 blocks x 64 cols

        # chunk -> idx column offsets (B group, C group, decode groups)
        co = 0
        blk_chunksB = [[] for _ in range(BPC)]
        for (k, r0, ch) in chunks:
            blk_chunksB[k].append((co, ch))
            co += (ch + 1) * 8
        blk_chunksC = [[] for _ in range(BPC)]
        for (k, r0, ch) in chunks:
            blk_chunksC[k].append((co, ch))
            co += (ch + 1) * 8
        aa_coffs = []
        for (r0, ch) in aa_chunks:
            aa_coffs.append(co)          # a-cols, then b-cols adjacent
            co += 2 * (ch + 1) * 8
        re_coffs = []
        for (r0, ch) in re_chunks:
            re_coffs.append(co)
            co += 2 * (ch + 1) * 8
        assert co == TOTW

        # ---------------- Phase A: replicated GEMM1 -> local hs table ----------------
        wrA = []
        with nc.named_scope("gemm1"):
            for j in range(C):                      # panel j: tabHS rows [j*NPC, (j+1)*NPC)
                for (o0, nb) in ((0, SPLIT_K), (SPLIT_O, BPC - SPLIT_K)):
                    x0 = xp_pool.tile([128, SPLIT_O], BF16, tag="x0", name="x0")
                    nc.sync.dma_start(out=x0[:, :nb * 128],
                                      in_=xT_d[0:128, j * NPC + o0: j * NPC + o0 + nb * 128])
                    x1 = xp_pool.tile([128, SPLIT_O], BF16, tag="x1", name="x1")
                    nc.sync.dma_start(out=x1[:, :nb * 128],
                                      in_=xT_d[128:256, j * NPC + o0: j * NPC + o0 + nb * 128])
                    if j == 0:
                        hpan = None
                    else:
                        hpan = hsb.tile([128, SPLIT_O], BF16, tag="hst", name="hst")
                    for q0 in range(0, nb, 4):
                        G = min(4, nb - q0)
                        ps = ppa.tile([128, 512], f32, tag="a", name="psa")
                        for g in range(G):
                            cs = slice((q0 + g) * 128, (q0 + g + 1) * 128)
                            nc.tensor.matmul(out=ps[:, g * 128:(g + 1) * 128],
                                             lhsT=x0[:, cs], rhs=W1_sb[0][:],
                                             start=True, stop=False)
                            nc.tensor.matmul(out=ps[:, g * 128:(g + 1) * 128],
                                             lhsT=x1[:, cs], rhs=W1_sb[1][:],
                                             start=False, stop=True)
                        if j == 0:
                            dst = hs_own[:, o0 + q0 * 128: o0 + (q0 + G) * 128]
                        else:
                            dst = hpan[:, q0 * 128:(q0 + G) * 128]
                        if (q0 // 4) % 2 == 0:
                            nc.scalar.activation(out=dst, in_=ps[:, :G * 128],
                                                 func=mybir.ActivationFunctionType.Copy)
                        else:
                            nc.vector.tensor_copy(out=dst, in_=ps[:, :G * 128])
                    # lane-major write: rows lane*NBLK + [b0, b0+nb)
                    b0 = j * BPC + o0 // 128
                    srct = hs_own[:, o0:o0 + nb * 128] if j == 0 else hpan[:, :nb * 128]
                    wrA.append(nc.sync.dma_start(
                        out=tabHS.rearrange("(l b) f -> l b f", b=NP // 128)[
                            :, b0:b0 + nb, :],
                        in_=srct.rearrange("p (b f) -> p b f", f=HID)))

        qctr = [0]

        def aggregate(k, tab, blk_chunks, own_tile, dep_ins):
            ps = pp.tile([128, HID], f32, tag="main", name="psagg")
            first = True
            for ci, (coff, ch) in enumerate(blk_chunks[k]):
                gt = gp.tile([128, CH_MAX + 1, HID], BF16, tag="gt", name="gt")
                qctr[0] = (qctr[0] + 1) % 4
                gi = nc.gpsimd.dma_gather(
                    gt[:, :ch + 1, :], tab[FBASE:, :],
                    idx_sb[:, coff:coff + (ch + 1) * 8],
                    (ch + 1) * 128, kreg[(ch + 1) * 128], HID, single_packet=False,
                    queue_num=qctr[0])
                for d in dep_ins:
                    add_dep_helper(gi.ins, d.ins, reason="gather after table ready")
                for r in range(ch):
                    nc.tensor.matmul(out=ps[:], lhsT=ident[:], rhs=gt[:, r, :],
                                     start=first, stop=False)
                    first = False
            nc.tensor.matmul(out=ps[:], lhsT=ident[:],
                             rhs=own_tile[:, k * 128:(k + 1) * 128],
                             start=first, stop=True)
            return ps

        # ---------------- Phase B: layer-1 aggregation (local) -> g ----------------
        wr2 = []
        ccs2 = []
        with nc.named_scope("agg1"):
            for k in range(BPC):
                dep = wrA if k == 0 else []
                ps = aggregate(k, tabHS, blk_chunksB, hs_own, dep)
                t1 = sp_.tile([128, HID], f32, tag="t1", name="t1")
                nc.scalar.activation(out=t1[:], in_=ps[:],
                                     func=mybir.ActivationFunctionType.Copy,
                                     scale=dinv[:, k:k + 1])
                t2 = sp_.tile([128, HID], f32, tag="t2", name="t2")
                nc.vector.tensor_add(out=t2[:], in0=t1[:], in1=bias1[:])
                nc.scalar.activation(out=g_own[:, k * 128:(k + 1) * 128], in_=t2[:],
                                     func=mybir.ActivationFunctionType.Relu,
                                     scale=dinv[:, k:k + 1])
                if k == SPLIT_K - 1:
                    w2a = nc.sync.dma_start(
                        out=ag2_in[0:HA, :].rearrange("(b l) f -> l b f", l=128),
                        in_=g_own[:, :HA].rearrange("p (b f) -> p b f", f=HID))
                    with nc.named_scope("ag2a"):
                        cc = nc.gpsimd.collective_compute(
                            "AllGather", mybir.AluOpType.bypass,
                            replica_groups=[list(range(C))],
                            ins=[ag2_in[0:HA, :].opt()], outs=[tab2[0:C * HA, :].opt()])
                        add_dep_helper(cc.ins, w2a.ins, reason="ag2a")
                        ccs2.append(cc)
            w2b = nc.sync.dma_start(
                out=ag2_in[HA:NPC, :].rearrange("(b l) f -> l b f", l=128),
                in_=g_own[:, HA:NPC].rearrange("p (b f) -> p b f", f=HID))
            with nc.named_scope("ag2b"):
                cc = nc.gpsimd.collective_compute(
                    "AllGather", mybir.AluOpType.bypass,
                    replica_groups=[list(range(C))],
                    ins=[ag2_in[HA:SPLIT_O2, :].opt()],
                    outs=[tab2[C * HA:C * SPLIT_O2, :].opt()])
                add_dep_helper(cc.ins, w2b.ins, reason="ag2b1")
                ccs2.append(cc)
                cc = nc.gpsimd.collective_compute(
                    "AllGather", mybir.AluOpType.bypass,
                    replica_groups=[list(range(C))],
                    ins=[ag2_in[SPLIT_O2:NPC, :].opt()],
                    outs=[tab2[C * SPLIT_O2:NP, :].opt()])
                add_dep_helper(cc.ins, w2b.ins, reason="ag2b2")
                ccs2.append(cc)

        # ---------------- Phase C: layer-2 aggregation + GEMM2 -> z ----------------
        wr3 = []
        ccs3 = []
        with nc.named_scope("agg2"):
            for k in range(BPC):
                dep = ccs2 if k == 0 else []
                ps = aggregate(k, tab2, blk_chunksC, g_own, dep)
                q = sp_.tile([128, HID], f32, tag="q", name="q")
                nc.scalar.activation(out=q[:], in_=ps[:],
                                     func=mybir.ActivationFunctionType.Copy,
                                     scale=dinv[:, k:k + 1])
                qt_ps = pz.tile([128, HID], f32, tag="qt", name="qtps")
                nc.tensor.transpose(out=qt_ps[:], in_=q[:], identity=identf[:])
                qt = sp_.tile([128, HID], BF16, tag="qt_sb", name="qtsb")
                nc.vector.tensor_copy(out=qt[:], in_=qt_ps[:])
                zps = pz.tile([128, OUT], f32, tag="z", name="zps")
                nc.tensor.matmul(out=zps[:], lhsT=qt[:], rhs=W2_sb[:], start=True, stop=True)
                nc.vector.tensor_add(out=z_own[:, k * OUT:(k + 1) * OUT],
                                     in0=zps[:], in1=bias2[:])
                if k == SPLIT_K - 1:
                    w3a = nc.sync.dma_start(
                        out=ag3_in[0:HA, :].rearrange("(b l) f -> l b f", l=128),
                        in_=z_own[:, :SPLIT_K * OUT].rearrange("p (b f) -> p b f", f=OUT))
                    with nc.named_scope("ag3a"):
                        cc = nc.gpsimd.collective_compute(
                            "AllGather", mybir.AluOpType.bypass,
                            replica_groups=[list(range(C))],
                            ins=[ag3_in[0:HA, :].opt()], outs=[ztab[0:C * HA, :].opt()])
                        add_dep_helper(cc.ins, w3a.ins, reason="ag3a")
                        ccs3.append(cc)
                if k == 40:
                    w3b1 = nc.sync.dma_start(
                        out=ag3_in[HA:SPLIT_O2, :].rearrange("(b l) f -> l b f", l=128),
                        in_=z_own[:, SPLIT_K * OUT:41 * OUT].rearrange(
                            "p (b f) -> p b f", f=OUT))
                    with nc.named_scope("ag3b1"):
                        cc = nc.gpsimd.collective_compute(
                            "AllGather", mybir.AluOpType.bypass,
                            replica_groups=[list(range(C))],
                            ins=[ag3_in[HA:SPLIT_O2, :].opt()],
                            outs=[ztab[C * HA:C * SPLIT_O2, :].opt()])
                        add_dep_helper(cc.ins, w3b1.ins, reason="ag3b1")
                        ccs3.append(cc)
            w3b2 = nc.sync.dma_start(
                out=ag3_in[SPLIT_O2:NPC, :].rearrange("(b l) f -> l b f", l=128),
                in_=z_own[:, 41 * OUT:].rearrange("p (b f) -> p b f", f=OUT))
            with nc.named_scope("ag3b"):
                cc = nc.gpsimd.collective_compute(
                    "AllGather", mybir.AluOpType.bypass,
                    replica_groups=[list(range(C))],
                    ins=[ag3_in[SPLIT_O2:NPC, :].opt()],
                    outs=[ztab[C * SPLIT_O2:NP, :].opt()])
                add_dep_helper(cc.ins, w3b2.ins, reason="ag3b2")
                ccs3.append(cc)

        # ---------------- Phase D: decode (AA after ag3a; rest after ag3b) ----------------
        with nc.named_scope("decode"):
            out_sb = own.tile([128, ndec_cols], f32)
            col = 0
            for grp_chunks, grp_coffs, grp_dep in (
                    (aa_chunks, aa_coffs, [ccs3[0]]),
                    (re_chunks, re_coffs, ccs3)):
                for i, (r0, ch) in enumerate(grp_chunks):
                    coff = grp_coffs[i]
                    qa = (2 * i) % 4
                    qb = (2 * i + 1) % 4
                    za = dgp.tile([128, DEC_CHUNK + 1, OUT], f32, tag="za", name="za")
                    ga = nc.gpsimd.dma_gather(
                        za[:, :ch + 1, :], ztab[FBASE:, :],
                        idx_sb[:, coff:coff + (ch + 1) * 8],
                        (ch + 1) * 128, kreg[(ch + 1) * 128], OUT, single_packet=False,
                        queue_num=qa)
                    zb = dgp.tile([128, DEC_CHUNK + 1, OUT], f32, tag="zb", name="zb")
                    gb = nc.gpsimd.dma_gather(
                        zb[:, :ch + 1, :], ztab[FBASE:, :],
                        idx_sb[:, coff + (ch + 1) * 8:coff + 2 * (ch + 1) * 8],
                        (ch + 1) * 128, kreg[(ch + 1) * 128], OUT, single_packet=False,
                        queue_num=qb)
                    for cc in grp_dep:
                        add_dep_helper(ga.ins, cc.ins, reason="decode after AG3")
                        add_dep_helper(gb.ins, cc.ins, reason="decode after AG3")
                    prod = prp.tile([128, ch * OUT], f32, tag="prod", name="prod")
                    nc.vector.tensor_mul(out=prod[:].rearrange("p (c o) -> p c o", o=OUT),
                                         in0=za[:, :ch, :], in1=zb[:, :ch, :])
                    nc.vector.reduce_sum(out=out_sb[:, col:col + ch],
                                         in_=prod[:].rearrange("p (c o) -> p c o", o=OUT),
                                         axis=mybir.AxisListType.X)
                    col += ch
            nc.sync.dma_start(out=out_d[:, :], in_=out_sb[:])

    lower_extended_insts(nc)
    _fix_sync_waits(nc)
    return nc


def kernel(x, W1, b1, W2, b2, edge_index, edge_label_index):
    x = np.asarray(x, dtype=np.float32)
    W1 = np.asarray(W1, dtype=np.float32)
    b1 = np.asarray(b1, dtype=np.float32)
    W2 = np.asarray(W2, dtype=np.float32)
    b2 = np.asarray(b2, dtype=np.float32)
    prep = _prepare(np.asarray(edge_index), np.asarray(edge_label_index))
    nc = _build(prep)

    # host-prescaled x: xp[pos] = dinv[pos] * x[perm[pos]]  (pads stay zero)
    xp = np.zeros((NP, IN), dtype=np.float32)
    rm = prep["real_mask"]
    xp[rm] = x[prep["final_perm"][rm]]
    dinv_full = 1.0 / np.sqrt(prep["degp"])
    xp *= dinv_full[:, None]

    import ml_dtypes

    def to_bf16(a):
        return np.asarray(a, dtype=np.float32).astype(ml_dtypes.bfloat16)

    in_maps = []
    for c in range(C):
        porder = prep["panel_orders"][c]
        xc = np.concatenate([xp[pc * NPC:(pc + 1) * NPC] for pc in porder], axis=0)
        in_maps.append({
            "xT": to_bf16(np.ascontiguousarray(xc.T)),
            "W1": to_bf16(W1), "b1": b1.reshape(1, HID),
            "W2": to_bf16(W2), "b2": b2.reshape(1, OUT),
            "degp": prep["degp_core"][c],
            "idx16": prep["idx16"][c],
        })
    res = run_bass_kernel_spmd(nc, in_maps, core_ids=list(range(C)))

    LPC = L // C
    out = np.empty(L, dtype=np.float32)
    for c in range(C):
        o = res.results[c]["out"]          # [128, ncols]; slot s at (s%128, s//128)
        sl = prep["dec_slots"][c]
        out[c * LPC:(c + 1) * LPC] = o[sl % 128, sl // 128]
    return out
